# revision 52
# baseline (speedup 1.0000x reference)
"""GATv2 block kernel for 8 Trainium2 NeuronCores (Bass/Tile).

Strategy (graph/data parallel over destination nodes):
  - Host sorts edges by destination, shards destination nodes across the
    8 cores, splits each core's nodes into tiles of 118 (padded into a
    128-partition frame: rows 118..127 carry the 10 edge-attr channels).
  - Per destination-node tile, edges are padded to multiples of 128
    ("chunks"); chunk counts per tile are maxed across cores so one SPMD
    program serves all 8 cores.
  - The per-edge data ships as ONE fp8 tensor ilv = [xsT | ITA] (two
    blocks along free): block0 = x[src]^T, block1 = fused node-indicator
    + attr rows.  Pass 1 computes s^T = w_l@xs^T + [xr|w_eT]@ITA in a
    SINGLE DoubleRow fp8 matmul per superchunk (virtual K=256, 2 fp8
    multiplies/cycle).  Prelu batches superchunk PAIRS (FD 1024 across
    two PSUM banks); m is written fp8 so the per-chunk logits matmuls
    get 4x fast-weight-load.
  - Pass 2 recomputes xl edge-major per chunk (fp8 stationary = ilv
    block0 chunk, bf16 w_l stream), DVE multiplies by exp(logits), and
    the scatter-matmul against the fp8 edge-major indicator It
    accumulates [sum(alpha*xl) | sum(exp)] in one 132-col psum stream.
  - Tail (SiLU + residual + LayerNorm) runs in bf16; normalization is a
    per-partition-scalar tensor_scalar (4x DVE mode).  Output is bf16.
"""

import numpy as np
import ml_dtypes

BF16 = ml_dtypes.bfloat16
FP8 = ml_dtypes.float8_e4m3fn

P = 128
NPT = 118   # destination nodes per tile (rows 118..127 carry attr)
HEADS = 4
HEAD_DIM = 32
OUT_DIM = 128
IN_DIM = 128
EDGE_DIM = 10
NEG_SLOPE = 0.2
LN_EPS = 1e-5
N_CORES = 8
SUPER = 4   # chunks per superchunk (free dim 512)
GRP = 7     # node tiles per tail group
PF = 5      # edge-load prefetch depth (tiles)

_CACHE = {}


def _build_program(C_list, trivial_affine):
    import concourse.bacc as bacc
    import concourse.tile as tile
    from concourse import mybir

    f32 = mybir.dt.float32
    bf16 = mybir.dt.bfloat16
    fp8 = mybir.dt.float8e4
    AT = mybir.ActivationFunctionType
    OP = mybir.AluOpType
    PM = mybir.MatmulPerfMode

    NT = len(C_list)                       # 53 node tiles per core
    CMAX = max(C_list)
    TOTAL_CHUNKS = sum(C_list)
    NN = NT * NPT                          # 6254 packed own nodes
    NNP = NN + (P - NPT)                   # stationary spill pad
    NP_PAD = NT * P                        # 6784 (tail frames, 118 used)
    EW = TOTAL_CHUNKS * P                  # padded edges per core
    NG = (NT + GRP - 1) // GRP             # tail groups
    EARLY_G = 6                            # groups normalized in first batch
    EARLY_T = 43                           # emit first normalize batch here
    EARLY_T2 = NT - 2                      # second batch (group EARLY_G) here
    CM4 = CMAX * HEADS                     # logits cols per tile slot

    assert 2 * CM4 + P <= 512, "pers psum bank overflow"

    nc = bacc.Bacc('TRN2', target_bir_lowering=False, debug=False,
                   enable_asserts=True, num_devices=N_CORES)

    # ---- external inputs ----
    ilv_d = nc.dram_tensor('ilv', [P, 2, EW], fp8, kind='ExternalInput')
    It_d = nc.dram_tensor('It_d', [P, EW], fp8, kind='ExternalInput')
    x_ownT = nc.dram_tensor('x_ownT', [P, NNP], fp8, kind='ExternalInput')
    x_own = nc.dram_tensor('x_own', [P, NP_PAD], bf16, kind='ExternalInput')
    ilv_par = nc.dram_tensor('ilv_par', [P, NT, 2, P], fp8,
                             kind='ExternalInput')
    # packed small consts: [w_lT (128) | w_rT (128) | att (4)] bf16
    wcat = nc.dram_tensor('wcat', [P, 2 * P + HEADS], bf16,
                          kind='ExternalInput')
    bias_lr = nc.dram_tensor('bias_lr', [P, 1], f32, kind='ExternalInput')
    aff = None
    if not trivial_affine:
        # rows: b_l bcast, conv_bias bcast, gamma bcast, beta bcast
        aff = nc.dram_tensor('aff', [P, 4 * P], f32, kind='ExternalInput')

    out_d = nc.dram_tensor('out', [P, NP_PAD], bf16, kind='ExternalOutput')

    # chunk start offsets per tile
    tile_chunk0 = []
    acc = 0
    for t in range(NT):
        tile_chunk0.append(acc)
        acc += C_list[t]

    with tile.TileContext(nc) as tc:
        with tc.tile_pool(name='const', bufs=1) as cp:
            # x_ownT arrives in 8 slices so phase 1 starts before the
            # whole tensor lands.
            # x_ownT/ilv_par arrive in slices; slice 0 up front, the rest
            # deferred into the loop so the startup DMA burst stays small
            c_xownT = cp.tile([P, NNP], fp8)
            NSL = 8
            sl = (NNP + NSL - 1) // NSL

            def emit_xownT_slice(i):
                a, b = i * sl, min(NNP, (i + 1) * sl)
                nc.scalar.dma_start(c_xownT[:, a:b], x_ownT[:, a:b])

            emit_xownT_slice(0)
            c_wcat = cp.tile([P, 2 * P + HEADS], bf16)
            nc.sync.dma_start(c_wcat[:], wcat[:])
            c_wlT = c_wcat[:, 0:P]
            c_wrT = c_wcat[:, P:2 * P]
            c_att = c_wcat[:, 2 * P:2 * P + HEADS]
            c_blr = cp.tile([P, 1], f32)
            nc.sync.dma_start(c_blr[:], bias_lr[:])
            c_aff = None
            if aff is not None:
                c_aff = cp.tile([P, 4 * P], f32)
                nc.sync.dma_start(c_aff[:], aff[:])

            with tc.tile_pool(name='persist', bufs=1) as pp:
                # [w_lT | (xr|w_eT)] fp8 stationary blocks per tile; odd
                # block rows 0..117 are filled with xr by phase 1.
                ilv_sb = pp.tile([P, NT, 2, P], fp8)
                NSP = 8
                slp = (NT + NSP - 1) // NSP

                def emit_ilvpar_slice(i):
                    a, b = i * slp, min(NT, (i + 1) * slp)
                    if a < b:
                        nc.gpsimd.dma_start(ilv_sb[:, a:b], ilv_par[:, a:b])

                emit_ilvpar_slice(0)
                hbuf = pp.tile([P, NT * P], bf16)        # post-residual h
                stats = pp.tile([P, NT * 2], f32)        # mean, var interleaved
                vinvb = pp.tile([P, NT], f32)            # 1/(var+eps)
                vepsb = pp.tile([P, NT], f32)            # var+eps

                with tc.tile_pool(name='eload', bufs=PF + 3) as lp, \
                     tc.tile_pool(name='ework', bufs=3) as wp, \
                     tc.tile_pool(name='ubufp', bufs=3) as up, \
                     tc.tile_pool(name='tailp', bufs=2) as tp:

                    loads = {}

                    def emit_loads(t):
                        te0 = tile_chunk0[t] * P
                        TW = C_list[t] * P
                        HW = (C_list[t] // 2) * P
                        ilv_t = lp.tile([P, 2, CMAX * P], fp8, tag='ilv')
                        # two halves so pass 1 starts before the whole
                        # tile's edges land
                        if HW:
                            nc.sync.dma_start(ilv_t[:, :, :HW],
                                              ilv_d[:, :, te0:te0 + HW])
                        nc.sync.dma_start(ilv_t[:, :, HW:TW],
                                          ilv_d[:, :, te0 + HW:te0 + TW])
                        It_t = lp.tile([P, CMAX * P], fp8, tag='It')
                        nc.gpsimd.dma_start(It_t[:, :TW],
                                            It_d[:, te0:te0 + TW])
                        loads[t] = (ilv_t, It_t)

                    # shallow initial prefetch (DMA bandwidth is the
                    # startup bottleneck); ramps to PF inside the loop
                    PF0 = 2
                    for t in range(min(PF0, NT)):
                        emit_loads(t)
                    next_load = min(PF0, NT)

                    # -------- tail: silu + residual + LN stats ----------
                    def tail_group(g):
                        g0 = g * GRP
                        gn = min(GRP, NT - g0)
                        GW = gn * P
                        ub = ubuf_g[:, :gn * 132].rearrange(
                            'p (t w) -> p t w', w=132)
                        # rv = 2*(denom+eps) so the reciprocal yields
                        # 0.5/denom and u/2 falls out of the mult directly
                        rv = tp.tile([P, GRP * HEADS], f32, tag='rv')
                        nc.vector.tensor_scalar(
                            out=rv[:, :gn * HEADS].rearrange(
                                'p (t h) -> p t h', h=HEADS),
                            in0=ub[:, :, P:P + HEADS],
                            scalar1=1e-16, scalar2=2.0,
                            op0=OP.add, op1=OP.mult)
                        rvi = tp.tile([P, GRP * HEADS], f32, tag='rvi')
                        nc.vector.reciprocal(rvi[:, :gn * HEADS],
                                             rv[:, :gn * HEADS])
                        uh = tp.tile([P, GRP * P], bf16, tag='uh')
                        rvi_v = (rvi[:, :gn * HEADS]
                                 .rearrange('p (t h) -> p t h', h=HEADS)
                                 [:, :, :, None]
                                 .to_broadcast([P, gn, HEADS, HEAD_DIM]))
                        nc.gpsimd.tensor_tensor(
                            out=uh[:, :GW].rearrange(
                                'p (t h d) -> p t h d', h=HEADS, d=HEAD_DIM),
                            in0=ub[:, :, :P].rearrange(
                                'p t (h d) -> p t h d', h=HEADS),
                            in1=rvi_v, op=OP.mult)
                        if c_aff is not None:
                            # c_aff row 1 holds conv_bias/2 (host pre-halved)
                            bcv = c_aff[:, P:2 * P][:, None, :].to_broadcast(
                                [P, gn, P])
                            nc.vector.tensor_tensor(
                                out=uh[:, :GW].rearrange(
                                    'p (t f) -> p t f', f=P),
                                in0=uh[:, :GW].rearrange(
                                    'p (t f) -> p t f', f=P),
                                in1=bcv, op=OP.add)
                        # silu(u) = uh*(1+tanh(uh)), uh = u/2  (tanh shares
                        # the exp table set -> no ACT table switch)
                        th = tp.tile([P, GRP * P], bf16, tag='th')
                        nc.scalar.activation(th[:, :GW], uh[:, :GW], AT.Tanh)
                        ss = tp.tile([P, GRP * P], bf16, tag='ss')
                        nc.vector.scalar_tensor_tensor(
                            out=ss[:, :GW], in0=th[:, :GW], scalar=1.0,
                            in1=uh[:, :GW], op0=OP.add, op1=OP.mult)
                        xo = tp.tile([P, GRP * P], bf16, tag='xo')
                        nc.scalar.dma_start(
                            xo[:, :GW], x_own[:, g0 * P:g0 * P + GW])
                        h_sl = hbuf[:, g0 * P:g0 * P + GW]
                        nc.gpsimd.tensor_tensor(
                            out=h_sl, in0=ss[:, :GW], in1=xo[:, :GW],
                            op=OP.add)
                        for i in range(gn):
                            tt_ = g0 + i
                            bs = tp.tile([P, 6], f32, tag='bs')
                            nc.vector.bn_stats(
                                bs[:], hbuf[:, tt_ * P:(tt_ + 1) * P])
                            nc.vector.bn_aggr(stats[:, tt_ * 2:tt_ * 2 + 2],
                                              bs[:])
                        var_v = (stats[:, g0 * 2:(g0 + gn) * 2]
                                 .rearrange('p (t k) -> p t k', k=2)[:, :, 1])
                        nc.vector.tensor_scalar(
                            out=vepsb[:, g0:g0 + gn], in0=var_v,
                            scalar1=LN_EPS, scalar2=None, op0=OP.add)
                        nc.vector.reciprocal(vinvb[:, g0:g0 + gn],
                                             vepsb[:, g0:g0 + gn])

                    def normalize(g, rstd_ap):
                        # per-tile (h - mu) * rstd via per-partition scalars
                        g0 = g * GRP
                        gn = min(GRP, NT - g0)
                        GW = gn * P
                        o = tp.tile([P, GRP * P], bf16, tag='o')
                        for i in range(gn):
                            tt_ = g0 + i
                            nc.vector.tensor_scalar(
                                out=o[:, i * P:(i + 1) * P],
                                in0=hbuf[:, tt_ * P:(tt_ + 1) * P],
                                scalar1=stats[:, tt_ * 2:tt_ * 2 + 1],
                                scalar2=rstd_ap[:, i:i + 1],
                                op0=OP.subtract, op1=OP.mult)
                        if c_aff is not None:
                            gv = c_aff[:, 2 * P:3 * P][:, None, :]\
                                .to_broadcast([P, gn, P])
                            nc.vector.tensor_tensor(
                                out=o[:, :GW].rearrange(
                                    'p (t f) -> p t f', f=P),
                                in0=o[:, :GW].rearrange(
                                    'p (t f) -> p t f', f=P),
                                in1=gv, op=OP.mult)
                            bv = c_aff[:, 3 * P:4 * P][:, None, :]\
                                .to_broadcast([P, gn, P])
                            nc.vector.tensor_tensor(
                                out=o[:, :GW].rearrange(
                                    'p (t f) -> p t f', f=P),
                                in0=o[:, :GW].rearrange(
                                    'p (t f) -> p t f', f=P),
                                in1=bv, op=OP.add)
                        nc.scalar.dma_start(
                            out_d[:, g0 * P:g0 * P + GW], o[:, :GW])

                    def emit_rstd(name, a, n, iters=3):
                        # rstd = rsqrt(var+eps) via DVE Newton iterations
                        # (avoids the ~2.7us ACT sqrt table swaps)
                        rstd = tp.tile([P, n], f32, tag=name, name=name)
                        nc.vector.tensor_scalar(
                            out=rstd[:], in0=vinvb[:, a:a + n], scalar1=1.0,
                            scalar2=0.5, op0=OP.add, op1=OP.mult)
                        for _ in range(iters):
                            nrt_ = tp.tile([P, n], f32, tag=name + '_t',
                                           name='nrt_')
                            nc.vector.tensor_tensor(
                                out=nrt_[:], in0=rstd[:], in1=rstd[:],
                                op=OP.mult)
                            nc.vector.tensor_tensor(
                                out=nrt_[:], in0=nrt_[:],
                                in1=vepsb[:, a:a + n], op=OP.mult)
                            nc.vector.tensor_scalar(
                                out=nrt_[:], in0=nrt_[:], scalar1=-0.5,
                                scalar2=1.5, op0=OP.mult, op1=OP.add)
                            nc.vector.tensor_tensor(
                                out=rstd[:], in0=rstd[:], in1=nrt_[:],
                                op=OP.mult)
                        return rstd

                    with tc.tile_pool(name='psA', bufs=2, space='PSUM') as psA, \
                         tc.tile_pool(name='psX', bufs=2, space='PSUM') as psX, \
                         tc.tile_pool(name='psP', bufs=1, space='PSUM') as psP, \
                         tc.tile_pool(name='psO', bufs=1, space='PSUM') as psO:
                        # persistent bank: 2 logits slots + phase-1 xr slot
                        pers = psP.tile([P, 512], f32)
                        P1LA = 3

                        def emit_p1(t):
                            ps1 = pers[:, 384:512]
                            nc.tensor.matmul(
                                ps1,
                                lhsT=c_xownT[:, t * NPT:t * NPT + P],
                                rhs=c_wrT[:], start=True, stop=True)
                            nc.scalar.copy(
                                ilv_sb[:NPT, t, 1, :], ps1[:NPT, :])

                        for t in range(min(P1LA, NT)):
                            emit_p1(t)
                        ubuf_g = None
                        rstdE_p = [None]
                        norm_cur = [0]
                        o_cur = [None]
                        xol_p = [None]
                        LG0 = (NG - 1) * GRP   # first tile of last group
                        TPT0 = max(0, (NG - 2) * GRP)  # per-tile tail start
                        # deferred scatter state: (It_t, msg, Ct)
                        dstate = {}

                        def tail_tile(tt):
                            # single-tile tail for the last group: keeps the
                            # post-loop serial chain to one tile's worth
                            ii = tt % GRP
                            ub1 = ubuf_g[:, ii * 132:(ii + 1) * 132]
                            rv1 = tp.tile([P, HEADS], f32, tag='rv1',
                                          name='rv1')
                            nc.vector.tensor_scalar(
                                out=rv1[:], in0=ub1[:, P:P + HEADS],
                                scalar1=1e-16, scalar2=2.0,
                                op0=OP.add, op1=OP.mult)
                            rvi1 = tp.tile([P, HEADS], f32, tag='rvi1',
                                           name='rvi1')
                            nc.vector.reciprocal(rvi1[:], rv1[:])
                            uh1 = tp.tile([P, P], bf16, tag='uh1',
                                          name='uh1')
                            nc.vector.tensor_tensor(
                                out=uh1[:].rearrange(
                                    'p (h d) -> p h d', h=HEADS),
                                in0=ub1[:, 0:P].rearrange(
                                    'p (h d) -> p h d', h=HEADS),
                                in1=rvi1[:, :, None].to_broadcast(
                                    [P, HEADS, HEAD_DIM]),
                                op=OP.mult)
                            th1 = tp.tile([P, P], bf16, tag='th1',
                                          name='th1')
                            nc.scalar.activation(th1[:], uh1[:], AT.Tanh)
                            ss1 = tp.tile([P, P], bf16, tag='ss1',
                                          name='ss1')
                            nc.vector.scalar_tensor_tensor(
                                out=ss1[:], in0=th1[:], scalar=1.0,
                                in1=uh1[:], op0=OP.add, op1=OP.mult)
                            h_sl = hbuf[:, tt * P:(tt + 1) * P]
                            xi = tt - TPT0
                            nc.vector.tensor_tensor(
                                out=h_sl, in0=ss1[:],
                                in1=xol_p[0][:, xi * P:(xi + 1) * P],
                                op=OP.add)
                            bs1 = tp.tile([P, 6], f32, tag='bs')
                            nc.vector.bn_stats(bs1[:], h_sl)
                            nc.vector.bn_aggr(stats[:, tt * 2:tt * 2 + 2],
                                              bs1[:])
                            nc.vector.tensor_scalar(
                                out=vepsb[:, tt:tt + 1],
                                in0=stats[:, tt * 2 + 1:tt * 2 + 2],
                                scalar1=LN_EPS, scalar2=None, op0=OP.add)
                            nc.vector.reciprocal(vinvb[:, tt:tt + 1],
                                                 vepsb[:, tt:tt + 1])

                        def emit_scatter_tile(j):
                            # scatter for tile j, two iterations late: its
                            # DVE multiply had two full tiles of slack, so
                            # the in-order PE queue never stalls here.
                            It_j, msg_j, Cj = dstate.pop(j)
                            ps_out = psO.tile([P, 132], f32, tag='out')
                            for c in range(Cj):
                                nc.tensor.matmul(
                                    ps_out[:],
                                    lhsT=It_j[:, c * P:(c + 1) * P],
                                    rhs=msg_j[:, c, :],
                                    start=(c == 0), stop=(c == Cj - 1))
                            nc.scalar.copy(
                                ubuf_g[:, (j % GRP) * 132:
                                       (j % GRP + 1) * 132],
                                ps_out[:])
                            if c_aff is None:
                                if j == TPT0:
                                    xol_p[0] = tp.tile(
                                        [P, (NT - TPT0) * P], bf16,
                                        tag='xol', name='xol')
                                    nc.scalar.dma_start(
                                        xol_p[0][:],
                                        x_own[:, TPT0 * P:NT * P])
                                if j >= TPT0:
                                    tail_tile(j)
                                elif j % GRP == GRP - 1:
                                    tail_group(j // GRP)
                            else:
                                if j % GRP == GRP - 1 or j == NT - 1:
                                    tail_group(j // GRP)
                            if j == EARLY_T:
                                # groups 0..EARLY_G-1 are long done; the
                                # normalize TS bursts are spread over the
                                # following iterations at ~5 tiles each
                                rstdE_p[0] = emit_rstd(
                                    'rstdE', 0, EARLY_G * GRP)
                            if rstdE_p[0] is not None and j > EARLY_T:
                                if c_aff is not None:
                                    g = j - EARLY_T - 1
                                    if g < EARLY_G:
                                        normalize(
                                            g, rstdE_p[0][:, g * GRP:
                                                          (g + 1) * GRP])
                                else:
                                    budget = 5
                                    while budget > 0 and \
                                            norm_cur[0] < EARLY_G * GRP:
                                        tt_ = norm_cur[0]
                                        gg, ii = tt_ // GRP, tt_ % GRP
                                        if ii == 0:
                                            o_cur[0] = tp.tile(
                                                [P, GRP * P], bf16,
                                                tag='o', name='o')
                                        nc.vector.tensor_scalar(
                                            out=o_cur[0][:, ii * P:
                                                         (ii + 1) * P],
                                            in0=hbuf[:, tt_ * P:
                                                     (tt_ + 1) * P],
                                            scalar1=stats[:, tt_ * 2:
                                                          tt_ * 2 + 1],
                                            scalar2=rstdE_p[0][:, tt_:
                                                               tt_ + 1],
                                            op0=OP.subtract, op1=OP.mult)
                                        if ii == GRP - 1:
                                            nc.scalar.dma_start(
                                                out_d[:, gg * GRP * P:
                                                      (gg + 1) * GRP * P],
                                                o_cur[0][:])
                                        norm_cur[0] += 1
                                        budget -= 1
                            if j == EARLY_T2 and NG > EARLY_G + 1:
                                g0m = EARLY_G * GRP
                                rstdM = emit_rstd('rstdM', g0m, GRP)
                                normalize(EARLY_G, rstdM[:])

                        for t in range(NT + 2):
                            j = t - 2
                            if j >= 0:
                                if j % GRP == 0:
                                    ubuf_g = up.tile([P, GRP * 132], bf16,
                                                     tag='ubuf')
                                emit_scatter_tile(j)
                            if t >= NT:
                                continue
                            if t >= 1 and (t - 1) % 3 == 0:
                                i = (t - 1) // 3 + 1
                                if i < NSL:
                                    emit_xownT_slice(i)
                                if i < NSP:
                                    emit_ilvpar_slice(i)
                            if t + P1LA < NT:
                                emit_p1(t + P1LA)
                            nl = 0
                            while next_load < min(NT, t + PF + 1) and nl < 2:
                                emit_loads(next_load)
                                next_load += 1
                                nl += 1
                            ilv_t, It_t = loads.pop(t)
                            Ct = C_list[t]
                            n_super = (Ct + SUPER - 1) // SUPER
                            n_pair = (n_super + 1) // 2
                            lgex = pers[:, (t % 2) * CM4:(t % 2) * CM4
                                        + Ct * HEADS]
                            msg = wp.tile([P, CMAX, 132], bf16, tag='msg', bufs=5)
                            ilv_w = ilv_sb[:, t]          # [P, 2, 128]

                            # pass 1: s^T per superchunk via ONE DoubleRow
                            # matmul; Prelu per superchunk PAIR; logits per
                            # chunk (emitted one pair late to hide ACT
                            # latency from the in-order PE queue).
                            def emit_logits(q, m_q, wq):
                                for jj in range(wq // P):
                                    jg = q * 2 * SUPER + jj
                                    nc.tensor.matmul(
                                        lgex[:, jg * HEADS:(jg + 1) * HEADS],
                                        lhsT=m_q[:, jj * P:(jj + 1) * P],
                                        rhs=c_att[:], start=True, stop=True)

                            pend1 = None
                            for q in range(n_pair):
                                ps_pair = psA.tile([P, 1024], f32, tag='sT')
                                wq = 0
                                for si in range(2):
                                    s = q * 2 + si
                                    if s >= n_super:
                                        break
                                    nch = min(SUPER, Ct - s * SUPER)
                                    W = nch * P
                                    o0 = s * SUPER * P
                                    nc.tensor.matmul(
                                        ps_pair[:, si * 512:si * 512 + W],
                                        lhsT=ilv_w,
                                        rhs=ilv_t[:, :, o0:o0 + W],
                                        start=True, stop=True,
                                        perf_mode=PM.DoubleRow)
                                    wq = si * 512 + W
                                m_q = wp.tile([P, 1024], fp8, tag='m')
                                nc.scalar.activation(
                                    m_q[:, :wq], ps_pair[:, :wq],
                                    AT.Prelu, bias=c_blr[:], alpha=NEG_SLOPE)
                                if pend1 is not None:
                                    emit_logits(*pend1)
                                pend1 = (q, m_q, wq)
                            emit_logits(*pend1)

                            # one Exp for the whole tile, into msg denom cols
                            nc.scalar.activation(
                                msg[:, :Ct, P:P + HEADS],
                                lgex.rearrange('p (c h) -> p c h', h=HEADS),
                                AT.Exp)

                            # pass 2: xl edge-major + alpha-weighting.
                            # multiply for super s emitted after xl of s+1.
                            def emit_mult(s, nch, ps_xl):
                                c0 = s * SUPER
                                W = nch * P
                                xl_v = ps_xl[:, :W].rearrange(
                                    'p (c f) -> p c f', c=nch)
                                if c_aff is not None:
                                    xl_sb = wp.tile([P, SUPER * P], bf16,
                                                    tag='xlb')
                                    blv = c_aff[:, 0:P][:, None, :]\
                                        .to_broadcast([P, nch, P])
                                    nc.vector.tensor_tensor(
                                        out=xl_sb[:, :W].rearrange(
                                            'p (c f) -> p c f', c=nch),
                                        in0=xl_v, in1=blv, op=OP.add)
                                    xl_v = xl_sb[:, :W].rearrange(
                                        'p (c f) -> p c f', c=nch)
                                ex_v = (msg[:, c0:c0 + nch, P:P + HEADS]
                                        [:, :, :, None].to_broadcast(
                                            [P, nch, HEADS, HEAD_DIM]))
                                nc.vector.tensor_tensor(
                                    out=msg[:, c0:c0 + nch, 0:P].rearrange(
                                        'p c (h d) -> p c h d', h=HEADS),
                                    in0=xl_v.rearrange(
                                        'p c (h d) -> p c h d', h=HEADS),
                                    in1=ex_v, op=OP.mult)

                            pend2 = None
                            for s in range(n_super):
                                nch = min(SUPER, Ct - s * SUPER)
                                o0 = s * SUPER * P
                                ps_xl = psX.tile([P, SUPER * P], f32,
                                                 tag='xl')
                                for jj in range(nch):
                                    nc.tensor.matmul(
                                        ps_xl[:, jj * P:(jj + 1) * P],
                                        lhsT=ilv_t[:, 0, o0 + jj * P:
                                                   o0 + (jj + 1) * P],
                                        rhs=c_wlT[:], start=True, stop=True)
                                if pend2 is not None:
                                    emit_mult(*pend2)
                                pend2 = (s, nch, ps_xl)
                            emit_mult(*pend2)
                            dstate[t] = (It_t, msg, Ct)

                        # ---------- end: remaining groups ----------
                        if c_aff is None and rstdE_p[0] is not None:
                            while norm_cur[0] < EARLY_G * GRP:
                                tt_ = norm_cur[0]
                                gg, ii = tt_ // GRP, tt_ % GRP
                                if ii == 0:
                                    o_cur[0] = tp.tile(
                                        [P, GRP * P], bf16,
                                        tag='o', name='o')
                                nc.vector.tensor_scalar(
                                    out=o_cur[0][:, ii * P:(ii + 1) * P],
                                    in0=hbuf[:, tt_ * P:(tt_ + 1) * P],
                                    scalar1=stats[:, tt_ * 2:tt_ * 2 + 1],
                                    scalar2=rstdE_p[0][:, tt_:tt_ + 1],
                                    op0=OP.subtract, op1=OP.mult)
                                if ii == GRP - 1:
                                    nc.scalar.dma_start(
                                        out_d[:, gg * GRP * P:
                                              (gg + 1) * GRP * P],
                                        o_cur[0][:])
                                norm_cur[0] += 1
                        if c_aff is None:
                            # ---------- end: last group (per-tile tails
                            # already done; rstd + normalize + store) ------
                            gn_l = NT - LG0
                            rstdL = emit_rstd('rstdL', LG0, gn_l, iters=2)
                            o_l = tp.tile([P, GRP * P], bf16, tag='o',
                                          name='o_l')
                            for i in range(gn_l):
                                tt_ = LG0 + i
                                nc.vector.tensor_scalar(
                                    out=o_l[:, i * P:(i + 1) * P],
                                    in0=hbuf[:, tt_ * P:(tt_ + 1) * P],
                                    scalar1=stats[:, tt_ * 2:tt_ * 2 + 1],
                                    scalar2=rstdL[:, i:i + 1],
                                    op0=OP.subtract, op1=OP.mult)
                            nc.scalar.dma_start(
                                out_d[:, LG0 * P:LG0 * P + gn_l * P],
                                o_l[:, :gn_l * P])
                        else:
                            done_g = EARLY_G + (1 if NG > EARLY_G + 1 else 0)
                            g0r = done_g * GRP
                            nrem = NT - g0r
                            rstdL = emit_rstd('rstdL', g0r, nrem, iters=2)
                            for g in range(done_g, NG):
                                g0 = g * GRP
                                gn = min(GRP, NT - g0)
                                normalize(g, rstdL[:, g0 - g0r:
                                                   g0 - g0r + gn])

    nc.compile()
    return nc


def kernel(x, edge_index, edge_attr, w_l, b_l, w_r, b_r, w_e, att,
           conv_bias, ln_gamma, ln_beta):
    from concourse.bass_utils import run_bass_kernel_spmd

    x = np.asarray(x, dtype=np.float32)
    edge_index = np.asarray(edge_index)
    edge_attr = np.asarray(edge_attr, dtype=np.float32)
    w_l = np.asarray(w_l, dtype=np.float32)
    b_l = np.asarray(b_l, dtype=np.float32)
    w_r = np.asarray(w_r, dtype=np.float32)
    b_r = np.asarray(b_r, dtype=np.float32)
    w_e = np.asarray(w_e, dtype=np.float32)
    att = np.asarray(att, dtype=np.float32)
    conv_bias = np.asarray(conv_bias, dtype=np.float32)
    ln_gamma = np.asarray(ln_gamma, dtype=np.float32)
    ln_beta = np.asarray(ln_beta, dtype=np.float32)

    N = x.shape[0]
    NTG = (N + NPT - 1) // NPT                  # 424 global dst tiles
    NT = (NTG + N_CORES - 1) // N_CORES         # 53 slots per core
    NN = NT * NPT                               # 6254 packed own nodes
    NNP = NN + (P - NPT)
    NP_PAD = NT * P                             # 6784

    src = edge_index[0].astype(np.int64)
    dst = edge_index[1].astype(np.int64)

    trivial_affine = (not b_l.any()) and (not conv_bias.any()) and \
        np.all(ln_gamma == 1.0) and (not ln_beta.any())

    # Balance the global dst tiles across cores: sort by chunk count and
    # deal groups of 8 so the per-slot max (which every core pays) is
    # tight.
    g_cnt = np.bincount(dst // NPT, minlength=NTG)
    g_chunks = np.maximum(1, (g_cnt + P - 1) // P)
    order_g = np.argsort(-g_chunks, kind='stable')
    assign = np.full((NT, N_CORES), -1, dtype=np.int64)
    assign.reshape(-1)[:NTG] = order_g
    core_of = np.full(NTG, -1, dtype=np.int64)
    slot_of = np.full(NTG, -1, dtype=np.int64)
    for s in range(NT):
        for k in range(N_CORES):
            g = assign[s, k]
            if g >= 0:
                core_of[g] = k
                slot_of[g] = s
    C_list = [int(max(1, max(g_chunks[g] for g in assign[s] if g >= 0)))
              for s in range(NT)]
    TOTAL_CHUNKS = sum(C_list)
    EW = TOTAL_CHUNKS * P

    g_e = dst // NPT
    core = core_of[g_e]
    order = np.lexsort((dst,))
    src_s, dst_s, core_s = src[order], dst[order], core[order]
    attr_s = edge_attr[order]
    tile_of_e = slot_of[g_e][order]
    dloc_e = (dst_s % NPT)

    key = (tuple(C_list), trivial_affine)
    if key in _CACHE:
        nc = _CACHE[key]
    else:
        nc = _build_program(C_list, trivial_affine)
        _CACHE[key] = nc

    # chunk start offsets per tile
    tile_chunk0 = np.zeros(NT, dtype=np.int64)
    acc = 0
    for t in range(NT):
        tile_chunk0[t] = acc
        acc += C_list[t]

    # consts shared by all cores
    # [w_lT | (zeros|w_eT)] fp8 per tile; xr rows filled on device
    ilv_par_h = np.zeros((P, NT, 2, P), dtype=FP8)
    ilv_par_h[:, :, 0, :] = w_l.T.astype(FP8)[:, None, :]
    ilv_par_h[NPT:, :, 1, :] = w_e.T.astype(FP8)[:, None, :]
    att_exp_h = np.zeros((P, HEADS), dtype=np.float32)
    for h in range(HEADS):
        att_exp_h[h * HEAD_DIM:(h + 1) * HEAD_DIM, h] = att[h]
    wcat_h = np.concatenate(
        [w_l.T, w_r.T, att_exp_h], axis=1).astype(BF16).copy()
    bias_lr_h = (b_l + b_r)[:, None].astype(np.float32).copy()
    aff_h = None
    if not trivial_affine:
        aff_h = np.concatenate([
            np.broadcast_to(b_l, (P, P)),
            np.broadcast_to(conv_bias * 0.5, (P, P)),
            np.broadcast_to(ln_gamma, (P, P)),
            np.broadcast_to(ln_beta, (P, P))], axis=1).astype(np.float32).copy()

    in_maps = []
    for k in range(N_CORES):
        sel = core_s == k
        ksrc, ktile, dloc = src_s[sel], tile_of_e[sel], dloc_e[sel]
        kattr = attr_s[sel]
        # position of each edge in the padded layout
        # edges already sorted by dst -> grouped by tile, in order
        pos = np.empty(len(ksrc), dtype=np.int64)
        for t in range(NT):
            tsel = ktile == t
            n_t = int(tsel.sum())
            base = tile_chunk0[t] * P
            pos[tsel] = base + np.arange(n_t)
        # fused fp8 edge tensor: block0 = x[src]^T, block1 = indicator+attr
        ilv_h = np.zeros((P, 2, EW), dtype=FP8)
        ilv_h[:, 0, pos] = x[ksrc].T.astype(FP8)
        ilv_h[dloc, 1, pos] = FP8(1.0)
        ilv_h[NPT:, 1, :][:, pos] = kattr.T.astype(FP8)
        # edge-major indicator
        It_h = np.zeros((P, EW), dtype=FP8)
        It_h[pos % P, (pos // P) * P + dloc] = FP8(1.0)

        # pack this core's (permuted) tiles' node features
        xk = np.zeros((NNP, P), dtype=np.float32)
        for s in range(NT):
            g = assign[s, k]
            if g < 0:
                continue
            n0 = g * NPT
            n1 = min(n0 + NPT, N)
            if n1 > n0:
                xk[s * NPT:s * NPT + (n1 - n0)] = x[n0:n1]
        # feature-major for phase 1
        x_ownT_h = np.ascontiguousarray(xk.T).astype(FP8)
        # partition-major tail frames: [p, t*128+f] = xk[t*118+p, f], p<118
        x_own_pm = np.zeros((P, NP_PAD), dtype=BF16)
        x_own_pm.reshape(P, NT, P)[:NPT] = \
            xk[:NN].reshape(NT, NPT, P).transpose(1, 0, 2).astype(BF16)
        im = {
            'ilv': ilv_h, 'It_d': It_h,
            'x_ownT': x_ownT_h, 'x_own': x_own_pm,
            'ilv_par': ilv_par_h,
            'wcat': wcat_h, 'bias_lr': bias_lr_h,
        }
        if aff_h is not None:
            im['aff'] = aff_h
        in_maps.append(im)

    res = run_bass_kernel_spmd(nc, in_maps, list(range(N_CORES)))
    out_full = np.zeros((N, P), dtype=np.float32)
    for k in range(N_CORES):
        o = res.results[k]['out']            # [P, NT*P] partition-major bf16
        o = o.astype(np.float32)
        o = o.reshape(P, NT, P)[:NPT].transpose(1, 0, 2).reshape(NN, P)
        for s in range(NT):
            g = assign[s, k]
            if g < 0:
                continue
            n0 = g * NPT
            n1 = min(n0 + NPT, N)
            if n1 > n0:
                out_full[n0:n1] = o[s * NPT:s * NPT + (n1 - n0)]
    return out_full


# revision 53
# speedup vs baseline: 1.1923x; 1.1923x over previous
"""GATv2 block kernel for 8 Trainium2 NeuronCores (Bass/Tile).

Strategy (graph/data parallel over destination nodes):
  - Host sorts edges by destination, shards destination nodes across the
    8 cores, splits each core's nodes into tiles of 118 (padded into a
    128-partition frame: rows 118..127 carry the 10 edge-attr channels).
  - Per destination-node tile, edges are padded to multiples of 128
    ("chunks"); chunk counts per tile are maxed across cores so one SPMD
    program serves all 8 cores.
  - The per-edge data ships as ONE fp8 tensor ilv = [xsT | ITA] (two
    blocks along free): block0 = x[src]^T, block1 = fused node-indicator
    + attr rows.  Pass 1 computes s^T = w_l@xs^T + [xr|w_eT]@ITA in a
    SINGLE DoubleRow fp8 matmul per superchunk (virtual K=256, 2 fp8
    multiplies/cycle).  Prelu batches superchunk PAIRS (FD 1024 across
    two PSUM banks); m is written fp8 so the per-chunk logits matmuls
    get 4x fast-weight-load.
  - Pass 2 recomputes xl edge-major per chunk (fp8 stationary = ilv
    block0 chunk, bf16 w_l stream), DVE multiplies by exp(logits), and
    the scatter-matmul against the fp8 edge-major indicator It
    accumulates [sum(alpha*xl) | sum(exp)] in one 132-col psum stream.
  - Tail (SiLU + residual + LayerNorm) runs in bf16; normalization is a
    per-partition-scalar tensor_scalar (4x DVE mode).  Output is bf16.
"""

import numpy as np
import ml_dtypes

BF16 = ml_dtypes.bfloat16
FP8 = ml_dtypes.float8_e4m3fn

P = 128
NPT = 118   # destination nodes per tile (rows 118..127 carry attr)
HEADS = 4
HEAD_DIM = 32
OUT_DIM = 128
IN_DIM = 128
EDGE_DIM = 10
NEG_SLOPE = 0.2
LN_EPS = 1e-5
N_CORES = 8
SUPER = 4   # chunks per superchunk (free dim 512)
GRP = 7     # node tiles per tail group
PF = 5      # edge-load prefetch depth (tiles)

_CACHE = {}


def _build_program(C_list, trivial_affine):
    import concourse.bacc as bacc
    import concourse.tile as tile
    from concourse import mybir

    f32 = mybir.dt.float32
    bf16 = mybir.dt.bfloat16
    fp8 = mybir.dt.float8e4
    AT = mybir.ActivationFunctionType
    OP = mybir.AluOpType
    PM = mybir.MatmulPerfMode

    NT = len(C_list)                       # 53 node tiles per core
    CMAX = max(C_list)
    TOTAL_CHUNKS = sum(C_list)
    NN = NT * NPT                          # 6254 packed own nodes
    NNP = NN + (P - NPT)                   # stationary spill pad
    NP_PAD = NT * P                        # 6784 (tail frames, 118 used)
    EW = TOTAL_CHUNKS * P                  # padded edges per core
    NG = (NT + GRP - 1) // GRP             # tail groups
    EARLY_G = 6                            # groups normalized in first batch
    EARLY_T = 43                           # emit first normalize batch here
    EARLY_T2 = NT - 2                      # second batch (group EARLY_G) here
    CM4 = CMAX * HEADS                     # logits cols per tile slot

    assert 2 * CM4 + P <= 512, "pers psum bank overflow"

    nc = bacc.Bacc('TRN2', target_bir_lowering=False, debug=False,
                   enable_asserts=True, num_devices=N_CORES)

    # ---- external inputs ----
    ilv_d = nc.dram_tensor('ilv', [P, 2, EW], fp8, kind='ExternalInput')
    It_d = nc.dram_tensor('It_d', [P, EW], fp8, kind='ExternalInput')
    x_ownT = nc.dram_tensor('x_ownT', [P, NNP], fp8, kind='ExternalInput')
    x_own = nc.dram_tensor('x_own', [P, NP_PAD], bf16, kind='ExternalInput')
    ilv_par = nc.dram_tensor('ilv_par', [P, NT, 2, P], fp8,
                             kind='ExternalInput')
    # packed small consts: [w_lT (128) | w_rT (128) | att (4)] bf16
    wcat = nc.dram_tensor('wcat', [P, 2 * P + HEADS], bf16,
                          kind='ExternalInput')
    bias_lr = nc.dram_tensor('bias_lr', [P, 1], f32, kind='ExternalInput')
    aff = None
    if not trivial_affine:
        # rows: b_l bcast, conv_bias bcast, gamma bcast, beta bcast
        aff = nc.dram_tensor('aff', [P, 4 * P], f32, kind='ExternalInput')

    out_d = nc.dram_tensor('out', [P, NP_PAD], bf16, kind='ExternalOutput')

    # chunk start offsets per tile
    tile_chunk0 = []
    acc = 0
    for t in range(NT):
        tile_chunk0.append(acc)
        acc += C_list[t]

    with tile.TileContext(nc) as tc:
        with tc.tile_pool(name='const', bufs=1) as cp:
            # x_ownT arrives in 8 slices so phase 1 starts before the
            # whole tensor lands.
            # x_ownT/ilv_par arrive in slices; slice 0 up front, the rest
            # deferred into the loop so the startup DMA burst stays small
            c_xownT = cp.tile([P, NNP], fp8)
            NSL = 8
            sl = (NNP + NSL - 1) // NSL

            def emit_xownT_slice(i):
                a, b = i * sl, min(NNP, (i + 1) * sl)
                nc.scalar.dma_start(c_xownT[:, a:b], x_ownT[:, a:b])

            emit_xownT_slice(0)
            c_wcat = cp.tile([P, 2 * P + HEADS], bf16)
            nc.sync.dma_start(c_wcat[:], wcat[:])
            c_wlT = c_wcat[:, 0:P]
            c_wrT = c_wcat[:, P:2 * P]
            c_att = c_wcat[:, 2 * P:2 * P + HEADS]
            c_blr = cp.tile([P, 1], f32)
            nc.sync.dma_start(c_blr[:], bias_lr[:])
            c_aff = None
            if aff is not None:
                c_aff = cp.tile([P, 4 * P], f32)
                nc.sync.dma_start(c_aff[:], aff[:])

            with tc.tile_pool(name='persist', bufs=1) as pp:
                # [w_lT | (xr|w_eT)] fp8 stationary blocks per tile; odd
                # block rows 0..117 are filled with xr by phase 1.
                ilv_sb = pp.tile([P, NT, 2, P], fp8)
                NSP = 8
                slp = (NT + NSP - 1) // NSP

                def emit_ilvpar_slice(i):
                    a, b = i * slp, min(NT, (i + 1) * slp)
                    if a < b:
                        nc.gpsimd.dma_start(ilv_sb[:, a:b], ilv_par[:, a:b])

                emit_ilvpar_slice(0)
                hbuf = pp.tile([P, NT * P], bf16)        # post-residual h
                stats = pp.tile([P, NT * 2], f32)        # mean, var interleaved
                vinvb = pp.tile([P, NT], f32)            # 1/(var+eps)
                vepsb = pp.tile([P, NT], f32)            # var+eps

                with tc.tile_pool(name='eload', bufs=PF + 3) as lp, \
                     tc.tile_pool(name='ework', bufs=3) as wp, \
                     tc.tile_pool(name='ubufp', bufs=2) as up, \
                     tc.tile_pool(name='tailp', bufs=2) as tp:

                    loads = {}

                    def emit_loads(t):
                        te0 = tile_chunk0[t] * P
                        TW = C_list[t] * P
                        HW = (C_list[t] // 2) * P
                        ilv_t = lp.tile([P, 2, CMAX * P], fp8, tag='ilv')
                        # two halves so pass 1 starts before the whole
                        # tile's edges land
                        if HW:
                            nc.sync.dma_start(ilv_t[:, :, :HW],
                                              ilv_d[:, :, te0:te0 + HW])
                        nc.sync.dma_start(ilv_t[:, :, HW:TW],
                                          ilv_d[:, :, te0 + HW:te0 + TW])
                        It_t = lp.tile([P, CMAX * P], fp8, tag='It')
                        nc.gpsimd.dma_start(It_t[:, :TW],
                                            It_d[:, te0:te0 + TW])
                        loads[t] = (ilv_t, It_t)

                    # shallow initial prefetch (DMA bandwidth is the
                    # startup bottleneck); ramps to PF inside the loop
                    PF0 = 2
                    for t in range(min(PF0, NT)):
                        emit_loads(t)
                    next_load = min(PF0, NT)

                    # -------- tail: silu + residual + LN stats ----------
                    def tail_group(g):
                        g0 = g * GRP
                        gn = min(GRP, NT - g0)
                        GW = gn * P
                        ub = ubuf_g[:, :gn * 132].rearrange(
                            'p (t w) -> p t w', w=132)
                        # rv = 2*(denom+eps) so the reciprocal yields
                        # 0.5/denom and u/2 falls out of the mult directly
                        rv = tp.tile([P, GRP * HEADS], f32, tag='rv')
                        nc.vector.tensor_scalar(
                            out=rv[:, :gn * HEADS].rearrange(
                                'p (t h) -> p t h', h=HEADS),
                            in0=ub[:, :, P:P + HEADS],
                            scalar1=1e-16, scalar2=2.0,
                            op0=OP.add, op1=OP.mult)
                        rvi = tp.tile([P, GRP * HEADS], f32, tag='rvi')
                        nc.vector.reciprocal(rvi[:, :gn * HEADS],
                                             rv[:, :gn * HEADS])
                        uh = tp.tile([P, GRP * P], bf16, tag='uh')
                        rvi_v = (rvi[:, :gn * HEADS]
                                 .rearrange('p (t h) -> p t h', h=HEADS)
                                 [:, :, :, None]
                                 .to_broadcast([P, gn, HEADS, HEAD_DIM]))
                        nc.gpsimd.tensor_tensor(
                            out=uh[:, :GW].rearrange(
                                'p (t h d) -> p t h d', h=HEADS, d=HEAD_DIM),
                            in0=ub[:, :, :P].rearrange(
                                'p t (h d) -> p t h d', h=HEADS),
                            in1=rvi_v, op=OP.mult)
                        if c_aff is not None:
                            # c_aff row 1 holds conv_bias/2 (host pre-halved)
                            bcv = c_aff[:, P:2 * P][:, None, :].to_broadcast(
                                [P, gn, P])
                            nc.vector.tensor_tensor(
                                out=uh[:, :GW].rearrange(
                                    'p (t f) -> p t f', f=P),
                                in0=uh[:, :GW].rearrange(
                                    'p (t f) -> p t f', f=P),
                                in1=bcv, op=OP.add)
                        # silu(u) = uh*(1+tanh(uh)), uh = u/2  (tanh shares
                        # the exp table set -> no ACT table switch)
                        th = tp.tile([P, GRP * P], bf16, tag='th')
                        nc.scalar.activation(th[:, :GW], uh[:, :GW], AT.Tanh)
                        ss = tp.tile([P, GRP * P], bf16, tag='ss')
                        nc.vector.scalar_tensor_tensor(
                            out=ss[:, :GW], in0=th[:, :GW], scalar=1.0,
                            in1=uh[:, :GW], op0=OP.add, op1=OP.mult)
                        xo = tp.tile([P, GRP * P], bf16, tag='xo')
                        nc.scalar.dma_start(
                            xo[:, :GW], x_own[:, g0 * P:g0 * P + GW])
                        h_sl = hbuf[:, g0 * P:g0 * P + GW]
                        nc.gpsimd.tensor_tensor(
                            out=h_sl, in0=ss[:, :GW], in1=xo[:, :GW],
                            op=OP.add)
                        for i in range(gn):
                            tt_ = g0 + i
                            bs = tp.tile([P, 6], f32, tag='bs')
                            nc.vector.bn_stats(
                                bs[:], hbuf[:, tt_ * P:(tt_ + 1) * P])
                            nc.vector.bn_aggr(stats[:, tt_ * 2:tt_ * 2 + 2],
                                              bs[:])
                        var_v = (stats[:, g0 * 2:(g0 + gn) * 2]
                                 .rearrange('p (t k) -> p t k', k=2)[:, :, 1])
                        nc.vector.tensor_scalar(
                            out=vepsb[:, g0:g0 + gn], in0=var_v,
                            scalar1=LN_EPS, scalar2=None, op0=OP.add)
                        nc.vector.reciprocal(vinvb[:, g0:g0 + gn],
                                             vepsb[:, g0:g0 + gn])

                    def normalize(g, rstd_ap):
                        # per-tile (h - mu) * rstd via per-partition scalars
                        g0 = g * GRP
                        gn = min(GRP, NT - g0)
                        GW = gn * P
                        o = tp.tile([P, GRP * P], bf16, tag='o')
                        for i in range(gn):
                            tt_ = g0 + i
                            nc.vector.tensor_scalar(
                                out=o[:, i * P:(i + 1) * P],
                                in0=hbuf[:, tt_ * P:(tt_ + 1) * P],
                                scalar1=stats[:, tt_ * 2:tt_ * 2 + 1],
                                scalar2=rstd_ap[:, i:i + 1],
                                op0=OP.subtract, op1=OP.mult)
                        if c_aff is not None:
                            gv = c_aff[:, 2 * P:3 * P][:, None, :]\
                                .to_broadcast([P, gn, P])
                            nc.vector.tensor_tensor(
                                out=o[:, :GW].rearrange(
                                    'p (t f) -> p t f', f=P),
                                in0=o[:, :GW].rearrange(
                                    'p (t f) -> p t f', f=P),
                                in1=gv, op=OP.mult)
                            bv = c_aff[:, 3 * P:4 * P][:, None, :]\
                                .to_broadcast([P, gn, P])
                            nc.vector.tensor_tensor(
                                out=o[:, :GW].rearrange(
                                    'p (t f) -> p t f', f=P),
                                in0=o[:, :GW].rearrange(
                                    'p (t f) -> p t f', f=P),
                                in1=bv, op=OP.add)
                        nc.scalar.dma_start(
                            out_d[:, g0 * P:g0 * P + GW], o[:, :GW])

                    def emit_rstd(name, a, n, iters=3):
                        # rstd = rsqrt(var+eps) via DVE Newton iterations
                        # (avoids the ~2.7us ACT sqrt table swaps)
                        rstd = tp.tile([P, n], f32, tag=name, name=name)
                        nc.vector.tensor_scalar(
                            out=rstd[:], in0=vinvb[:, a:a + n], scalar1=1.0,
                            scalar2=0.5, op0=OP.add, op1=OP.mult)
                        for _ in range(iters):
                            nrt_ = tp.tile([P, n], f32, tag=name + '_t',
                                           name='nrt_')
                            nc.vector.tensor_tensor(
                                out=nrt_[:], in0=rstd[:], in1=rstd[:],
                                op=OP.mult)
                            nc.vector.tensor_tensor(
                                out=nrt_[:], in0=nrt_[:],
                                in1=vepsb[:, a:a + n], op=OP.mult)
                            nc.vector.tensor_scalar(
                                out=nrt_[:], in0=nrt_[:], scalar1=-0.5,
                                scalar2=1.5, op0=OP.mult, op1=OP.add)
                            nc.vector.tensor_tensor(
                                out=rstd[:], in0=rstd[:], in1=nrt_[:],
                                op=OP.mult)
                        return rstd

                    with tc.tile_pool(name='psA', bufs=2, space='PSUM') as psA, \
                         tc.tile_pool(name='psX', bufs=2, space='PSUM') as psX, \
                         tc.tile_pool(name='psP', bufs=1, space='PSUM') as psP, \
                         tc.tile_pool(name='psO', bufs=1, space='PSUM') as psO:
                        # persistent bank: 2 logits slots + phase-1 xr slot
                        pers = psP.tile([P, 512], f32)
                        P1LA = 3

                        def emit_p1(t):
                            ps1 = pers[:, 384:512]
                            nc.tensor.matmul(
                                ps1,
                                lhsT=c_xownT[:, t * NPT:t * NPT + P],
                                rhs=c_wrT[:], start=True, stop=True)
                            nc.scalar.copy(
                                ilv_sb[:NPT, t, 1, :], ps1[:NPT, :])

                        for t in range(min(P1LA, NT)):
                            emit_p1(t)
                        ubuf_g = None
                        rstdE_p = [None]
                        norm_cur = [0]
                        o_cur = [None]
                        xol_p = [None]
                        LG0 = (NG - 1) * GRP   # first tile of last group
                        TPT0 = max(0, (NG - 2) * GRP)  # per-tile tail start
                        # deferred scatter state: (It_t, msg, Ct)
                        dstate = {}

                        def tail_tile(tt):
                            # single-tile tail for the last group: keeps the
                            # post-loop serial chain to one tile's worth
                            ii = tt % GRP
                            ub1 = ubuf_g[:, ii * 132:(ii + 1) * 132]
                            rv1 = tp.tile([P, HEADS], f32, tag='rv1',
                                          name='rv1')
                            nc.vector.tensor_scalar(
                                out=rv1[:], in0=ub1[:, P:P + HEADS],
                                scalar1=1e-16, scalar2=2.0,
                                op0=OP.add, op1=OP.mult)
                            rvi1 = tp.tile([P, HEADS], f32, tag='rvi1',
                                           name='rvi1')
                            nc.vector.reciprocal(rvi1[:], rv1[:])
                            uh1 = tp.tile([P, P], bf16, tag='uh1',
                                          name='uh1')
                            nc.vector.tensor_tensor(
                                out=uh1[:].rearrange(
                                    'p (h d) -> p h d', h=HEADS),
                                in0=ub1[:, 0:P].rearrange(
                                    'p (h d) -> p h d', h=HEADS),
                                in1=rvi1[:, :, None].to_broadcast(
                                    [P, HEADS, HEAD_DIM]),
                                op=OP.mult)
                            th1 = tp.tile([P, P], bf16, tag='th1',
                                          name='th1')
                            nc.scalar.activation(th1[:], uh1[:], AT.Tanh)
                            ss1 = tp.tile([P, P], bf16, tag='ss1',
                                          name='ss1')
                            nc.vector.scalar_tensor_tensor(
                                out=ss1[:], in0=th1[:], scalar=1.0,
                                in1=uh1[:], op0=OP.add, op1=OP.mult)
                            h_sl = hbuf[:, tt * P:(tt + 1) * P]
                            xi = tt - TPT0
                            nc.vector.tensor_tensor(
                                out=h_sl, in0=ss1[:],
                                in1=xol_p[0][:, xi * P:(xi + 1) * P],
                                op=OP.add)
                            bs1 = tp.tile([P, 6], f32, tag='bs')
                            nc.vector.bn_stats(bs1[:], h_sl)
                            nc.vector.bn_aggr(stats[:, tt * 2:tt * 2 + 2],
                                              bs1[:])
                            nc.vector.tensor_scalar(
                                out=vepsb[:, tt:tt + 1],
                                in0=stats[:, tt * 2 + 1:tt * 2 + 2],
                                scalar1=LN_EPS, scalar2=None, op0=OP.add)
                            nc.vector.reciprocal(vinvb[:, tt:tt + 1],
                                                 vepsb[:, tt:tt + 1])

                        def emit_scatter_tile(j):
                            # scatter for tile j, two iterations late: its
                            # DVE multiply had two full tiles of slack, so
                            # the in-order PE queue never stalls here.
                            It_j, msg_j, Cj = dstate.pop(j)
                            ps_out = psO.tile([P, 132], f32, tag='out')
                            for c in range(Cj):
                                nc.tensor.matmul(
                                    ps_out[:],
                                    lhsT=It_j[:, c * P:(c + 1) * P],
                                    rhs=msg_j[:, c, :],
                                    start=(c == 0), stop=(c == Cj - 1))
                            nc.scalar.copy(
                                ubuf_g[:, (j % GRP) * 132:
                                       (j % GRP + 1) * 132],
                                ps_out[:])
                            if c_aff is None:
                                if j == TPT0:
                                    xol_p[0] = tp.tile(
                                        [P, (NT - TPT0) * P], bf16,
                                        tag='xol', name='xol')
                                    nc.scalar.dma_start(
                                        xol_p[0][:],
                                        x_own[:, TPT0 * P:NT * P])
                                if j >= TPT0:
                                    tail_tile(j)
                                elif j % GRP == GRP - 1:
                                    tail_group(j // GRP)
                            else:
                                if j % GRP == GRP - 1 or j == NT - 1:
                                    tail_group(j // GRP)
                            if j == EARLY_T:
                                # groups 0..EARLY_G-1 are long done; the
                                # normalize TS bursts are spread over the
                                # following iterations at ~5 tiles each
                                rstdE_p[0] = emit_rstd(
                                    'rstdE', 0, EARLY_G * GRP)
                            if rstdE_p[0] is not None and j > EARLY_T:
                                if c_aff is not None:
                                    g = j - EARLY_T - 1
                                    if g < EARLY_G:
                                        normalize(
                                            g, rstdE_p[0][:, g * GRP:
                                                          (g + 1) * GRP])
                                else:
                                    budget = 5
                                    while budget > 0 and \
                                            norm_cur[0] < EARLY_G * GRP:
                                        tt_ = norm_cur[0]
                                        gg, ii = tt_ // GRP, tt_ % GRP
                                        if ii == 0:
                                            o_cur[0] = tp.tile(
                                                [P, GRP * P], bf16,
                                                tag='o', name='o')
                                        nc.vector.tensor_scalar(
                                            out=o_cur[0][:, ii * P:
                                                         (ii + 1) * P],
                                            in0=hbuf[:, tt_ * P:
                                                     (tt_ + 1) * P],
                                            scalar1=stats[:, tt_ * 2:
                                                          tt_ * 2 + 1],
                                            scalar2=rstdE_p[0][:, tt_:
                                                               tt_ + 1],
                                            op0=OP.subtract, op1=OP.mult)
                                        if ii == GRP - 1:
                                            nc.scalar.dma_start(
                                                out_d[:, gg * GRP * P:
                                                      (gg + 1) * GRP * P],
                                                o_cur[0][:])
                                        norm_cur[0] += 1
                                        budget -= 1
                            if j == EARLY_T2 and NG > EARLY_G + 1:
                                g0m = EARLY_G * GRP
                                rstdM = emit_rstd('rstdM', g0m, GRP)
                                normalize(EARLY_G, rstdM[:])

                        for t in range(NT + 2):
                            j = t - 2
                            if j >= 0:
                                if j % GRP == 0:
                                    ubuf_g = up.tile([P, GRP * 132], bf16,
                                                     tag='ubuf')
                                emit_scatter_tile(j)
                            if t >= NT:
                                continue
                            if t >= 1 and (t - 1) % 3 == 0:
                                i = (t - 1) // 3 + 1
                                if i < NSL:
                                    emit_xownT_slice(i)
                                if i < NSP:
                                    emit_ilvpar_slice(i)
                            if t + P1LA < NT:
                                emit_p1(t + P1LA)
                            nl = 0
                            while next_load < min(NT, t + PF + 1) and nl < 2:
                                emit_loads(next_load)
                                next_load += 1
                                nl += 1
                            ilv_t, It_t = loads.pop(t)
                            Ct = C_list[t]
                            n_super = (Ct + SUPER - 1) // SUPER
                            n_pair = (n_super + 1) // 2
                            lgex = pers[:, (t % 2) * CM4:(t % 2) * CM4
                                        + Ct * HEADS]
                            msg = wp.tile([P, CMAX, 132], bf16, tag='msg', bufs=4)
                            ilv_w = ilv_sb[:, t]          # [P, 2, 128]

                            # pass 1: s^T per superchunk via ONE DoubleRow
                            # matmul; Prelu per superchunk PAIR; logits per
                            # chunk (emitted one pair late to hide ACT
                            # latency from the in-order PE queue).
                            def emit_logits(q, m_q, wq):
                                for jj in range(wq // P):
                                    jg = q * 2 * SUPER + jj
                                    nc.tensor.matmul(
                                        lgex[:, jg * HEADS:(jg + 1) * HEADS],
                                        lhsT=m_q[:, jj * P:(jj + 1) * P],
                                        rhs=c_att[:], start=True, stop=True)

                            pend1 = None
                            for q in range(n_pair):
                                ps_pair = psA.tile([P, 1024], f32, tag='sT')
                                wq = 0
                                for si in range(2):
                                    s = q * 2 + si
                                    if s >= n_super:
                                        break
                                    nch = min(SUPER, Ct - s * SUPER)
                                    W = nch * P
                                    o0 = s * SUPER * P
                                    nc.tensor.matmul(
                                        ps_pair[:, si * 512:si * 512 + W],
                                        lhsT=ilv_w,
                                        rhs=ilv_t[:, :, o0:o0 + W],
                                        start=True, stop=True,
                                        perf_mode=PM.DoubleRow)
                                    wq = si * 512 + W
                                m_q = wp.tile([P, 1024], fp8, tag='m')
                                nc.scalar.activation(
                                    m_q[:, :wq], ps_pair[:, :wq],
                                    AT.Prelu, bias=c_blr[:], alpha=NEG_SLOPE)
                                if pend1 is not None:
                                    emit_logits(*pend1)
                                pend1 = (q, m_q, wq)
                            emit_logits(*pend1)

                            # one Exp for the whole tile, into msg denom cols
                            nc.scalar.activation(
                                msg[:, :Ct, P:P + HEADS],
                                lgex.rearrange('p (c h) -> p c h', h=HEADS),
                                AT.Exp)

                            # pass 2: xl edge-major + alpha-weighting.
                            # multiply for super s emitted after xl of s+1.
                            def emit_mult(s, nch, ps_xl):
                                c0 = s * SUPER
                                W = nch * P
                                xl_v = ps_xl[:, :W].rearrange(
                                    'p (c f) -> p c f', c=nch)
                                if c_aff is not None:
                                    xl_sb = wp.tile([P, SUPER * P], bf16,
                                                    tag='xlb')
                                    blv = c_aff[:, 0:P][:, None, :]\
                                        .to_broadcast([P, nch, P])
                                    nc.vector.tensor_tensor(
                                        out=xl_sb[:, :W].rearrange(
                                            'p (c f) -> p c f', c=nch),
                                        in0=xl_v, in1=blv, op=OP.add)
                                    xl_v = xl_sb[:, :W].rearrange(
                                        'p (c f) -> p c f', c=nch)
                                ex_v = (msg[:, c0:c0 + nch, P:P + HEADS]
                                        [:, :, :, None].to_broadcast(
                                            [P, nch, HEADS, HEAD_DIM]))
                                nc.vector.tensor_tensor(
                                    out=msg[:, c0:c0 + nch, 0:P].rearrange(
                                        'p c (h d) -> p c h d', h=HEADS),
                                    in0=xl_v.rearrange(
                                        'p c (h d) -> p c h d', h=HEADS),
                                    in1=ex_v, op=OP.mult)

                            pend2 = None
                            for s in range(n_super):
                                nch = min(SUPER, Ct - s * SUPER)
                                o0 = s * SUPER * P
                                ps_xl = psX.tile([P, SUPER * P], f32,
                                                 tag='xl')
                                for jj in range(nch):
                                    nc.tensor.matmul(
                                        ps_xl[:, jj * P:(jj + 1) * P],
                                        lhsT=ilv_t[:, 0, o0 + jj * P:
                                                   o0 + (jj + 1) * P],
                                        rhs=c_wlT[:], start=True, stop=True)
                                if pend2 is not None:
                                    emit_mult(*pend2)
                                pend2 = (s, nch, ps_xl)
                            emit_mult(*pend2)
                            dstate[t] = (It_t, msg, Ct)

                        # ---------- end: remaining groups ----------
                        if c_aff is None and rstdE_p[0] is not None:
                            while norm_cur[0] < EARLY_G * GRP:
                                tt_ = norm_cur[0]
                                gg, ii = tt_ // GRP, tt_ % GRP
                                if ii == 0:
                                    o_cur[0] = tp.tile(
                                        [P, GRP * P], bf16,
                                        tag='o', name='o')
                                nc.vector.tensor_scalar(
                                    out=o_cur[0][:, ii * P:(ii + 1) * P],
                                    in0=hbuf[:, tt_ * P:(tt_ + 1) * P],
                                    scalar1=stats[:, tt_ * 2:tt_ * 2 + 1],
                                    scalar2=rstdE_p[0][:, tt_:tt_ + 1],
                                    op0=OP.subtract, op1=OP.mult)
                                if ii == GRP - 1:
                                    nc.scalar.dma_start(
                                        out_d[:, gg * GRP * P:
                                              (gg + 1) * GRP * P],
                                        o_cur[0][:])
                                norm_cur[0] += 1
                        if c_aff is None:
                            # ---------- end: last group (per-tile tails
                            # already done; rstd + normalize + store) ------
                            gn_l = NT - LG0
                            rstdL = emit_rstd('rstdL', LG0, gn_l, iters=2)
                            o_l = tp.tile([P, GRP * P], bf16, tag='o',
                                          name='o_l')
                            for i in range(gn_l):
                                tt_ = LG0 + i
                                nc.vector.tensor_scalar(
                                    out=o_l[:, i * P:(i + 1) * P],
                                    in0=hbuf[:, tt_ * P:(tt_ + 1) * P],
                                    scalar1=stats[:, tt_ * 2:tt_ * 2 + 1],
                                    scalar2=rstdL[:, i:i + 1],
                                    op0=OP.subtract, op1=OP.mult)
                            nc.scalar.dma_start(
                                out_d[:, LG0 * P:LG0 * P + gn_l * P],
                                o_l[:, :gn_l * P])
                        else:
                            done_g = EARLY_G + (1 if NG > EARLY_G + 1 else 0)
                            g0r = done_g * GRP
                            nrem = NT - g0r
                            rstdL = emit_rstd('rstdL', g0r, nrem, iters=2)
                            for g in range(done_g, NG):
                                g0 = g * GRP
                                gn = min(GRP, NT - g0)
                                normalize(g, rstdL[:, g0 - g0r:
                                                   g0 - g0r + gn])

    nc.compile()
    return nc


def kernel(x, edge_index, edge_attr, w_l, b_l, w_r, b_r, w_e, att,
           conv_bias, ln_gamma, ln_beta):
    from concourse.bass_utils import run_bass_kernel_spmd

    x = np.asarray(x, dtype=np.float32)
    edge_index = np.asarray(edge_index)
    edge_attr = np.asarray(edge_attr, dtype=np.float32)
    w_l = np.asarray(w_l, dtype=np.float32)
    b_l = np.asarray(b_l, dtype=np.float32)
    w_r = np.asarray(w_r, dtype=np.float32)
    b_r = np.asarray(b_r, dtype=np.float32)
    w_e = np.asarray(w_e, dtype=np.float32)
    att = np.asarray(att, dtype=np.float32)
    conv_bias = np.asarray(conv_bias, dtype=np.float32)
    ln_gamma = np.asarray(ln_gamma, dtype=np.float32)
    ln_beta = np.asarray(ln_beta, dtype=np.float32)

    N = x.shape[0]
    NTG = (N + NPT - 1) // NPT                  # 424 global dst tiles
    NT = (NTG + N_CORES - 1) // N_CORES         # 53 slots per core
    NN = NT * NPT                               # 6254 packed own nodes
    NNP = NN + (P - NPT)
    NP_PAD = NT * P                             # 6784

    src = edge_index[0].astype(np.int64)
    dst = edge_index[1].astype(np.int64)

    trivial_affine = (not b_l.any()) and (not conv_bias.any()) and \
        np.all(ln_gamma == 1.0) and (not ln_beta.any())

    # Balance the global dst tiles across cores: sort by chunk count and
    # deal groups of 8 so the per-slot max (which every core pays) is
    # tight.
    g_cnt = np.bincount(dst // NPT, minlength=NTG)
    g_chunks = np.maximum(1, (g_cnt + P - 1) // P)
    order_g = np.argsort(-g_chunks, kind='stable')
    assign = np.full((NT, N_CORES), -1, dtype=np.int64)
    assign.reshape(-1)[:NTG] = order_g
    core_of = np.full(NTG, -1, dtype=np.int64)
    slot_of = np.full(NTG, -1, dtype=np.int64)
    for s in range(NT):
        for k in range(N_CORES):
            g = assign[s, k]
            if g >= 0:
                core_of[g] = k
                slot_of[g] = s
    C_list = [int(max(1, max(g_chunks[g] for g in assign[s] if g >= 0)))
              for s in range(NT)]
    TOTAL_CHUNKS = sum(C_list)
    EW = TOTAL_CHUNKS * P

    g_e = dst // NPT
    core = core_of[g_e]
    order = np.lexsort((dst,))
    src_s, dst_s, core_s = src[order], dst[order], core[order]
    attr_s = edge_attr[order]
    tile_of_e = slot_of[g_e][order]
    dloc_e = (dst_s % NPT)

    key = (tuple(C_list), trivial_affine)
    if key in _CACHE:
        nc = _CACHE[key]
    else:
        nc = _build_program(C_list, trivial_affine)
        _CACHE[key] = nc

    # chunk start offsets per tile
    tile_chunk0 = np.zeros(NT, dtype=np.int64)
    acc = 0
    for t in range(NT):
        tile_chunk0[t] = acc
        acc += C_list[t]

    # consts shared by all cores
    # [w_lT | (zeros|w_eT)] fp8 per tile; xr rows filled on device
    ilv_par_h = np.zeros((P, NT, 2, P), dtype=FP8)
    ilv_par_h[:, :, 0, :] = w_l.T.astype(FP8)[:, None, :]
    ilv_par_h[NPT:, :, 1, :] = w_e.T.astype(FP8)[:, None, :]
    att_exp_h = np.zeros((P, HEADS), dtype=np.float32)
    for h in range(HEADS):
        att_exp_h[h * HEAD_DIM:(h + 1) * HEAD_DIM, h] = att[h]
    wcat_h = np.concatenate(
        [w_l.T, w_r.T, att_exp_h], axis=1).astype(BF16).copy()
    bias_lr_h = (b_l + b_r)[:, None].astype(np.float32).copy()
    aff_h = None
    if not trivial_affine:
        aff_h = np.concatenate([
            np.broadcast_to(b_l, (P, P)),
            np.broadcast_to(conv_bias * 0.5, (P, P)),
            np.broadcast_to(ln_gamma, (P, P)),
            np.broadcast_to(ln_beta, (P, P))], axis=1).astype(np.float32).copy()

    in_maps = []
    for k in range(N_CORES):
        sel = core_s == k
        ksrc, ktile, dloc = src_s[sel], tile_of_e[sel], dloc_e[sel]
        kattr = attr_s[sel]
        # position of each edge in the padded layout
        # edges already sorted by dst -> grouped by tile, in order
        pos = np.empty(len(ksrc), dtype=np.int64)
        for t in range(NT):
            tsel = ktile == t
            n_t = int(tsel.sum())
            base = tile_chunk0[t] * P
            pos[tsel] = base + np.arange(n_t)
        # fused fp8 edge tensor: block0 = x[src]^T, block1 = indicator+attr
        ilv_h = np.zeros((P, 2, EW), dtype=FP8)
        ilv_h[:, 0, pos] = x[ksrc].T.astype(FP8)
        ilv_h[dloc, 1, pos] = FP8(1.0)
        ilv_h[NPT:, 1, :][:, pos] = kattr.T.astype(FP8)
        # edge-major indicator
        It_h = np.zeros((P, EW), dtype=FP8)
        It_h[pos % P, (pos // P) * P + dloc] = FP8(1.0)

        # pack this core's (permuted) tiles' node features
        xk = np.zeros((NNP, P), dtype=np.float32)
        for s in range(NT):
            g = assign[s, k]
            if g < 0:
                continue
            n0 = g * NPT
            n1 = min(n0 + NPT, N)
            if n1 > n0:
                xk[s * NPT:s * NPT + (n1 - n0)] = x[n0:n1]
        # feature-major for phase 1
        x_ownT_h = np.ascontiguousarray(xk.T).astype(FP8)
        # partition-major tail frames: [p, t*128+f] = xk[t*118+p, f], p<118
        x_own_pm = np.zeros((P, NP_PAD), dtype=BF16)
        x_own_pm.reshape(P, NT, P)[:NPT] = \
            xk[:NN].reshape(NT, NPT, P).transpose(1, 0, 2).astype(BF16)
        im = {
            'ilv': ilv_h, 'It_d': It_h,
            'x_ownT': x_ownT_h, 'x_own': x_own_pm,
            'ilv_par': ilv_par_h,
            'wcat': wcat_h, 'bias_lr': bias_lr_h,
        }
        if aff_h is not None:
            im['aff'] = aff_h
        in_maps.append(im)

    res = run_bass_kernel_spmd(nc, in_maps, list(range(N_CORES)))
    out_full = np.zeros((N, P), dtype=np.float32)
    for k in range(N_CORES):
        o = res.results[k]['out']            # [P, NT*P] partition-major bf16
        o = o.astype(np.float32)
        o = o.reshape(P, NT, P)[:NPT].transpose(1, 0, 2).reshape(NN, P)
        for s in range(NT):
            g = assign[s, k]
            if g < 0:
                continue
            n0 = g * NPT
            n1 = min(n0 + NPT, N)
            if n1 > n0:
                out_full[n0:n1] = o[s * NPT:s * NPT + (n1 - n0)]
    return out_full
